# revision 1
# baseline (speedup 1.0000x reference)
"""Trainium2 Bass kernel for a GPT-2 style transformer block.

Sharding (8 NeuronCores, SPMD-uniform program):
  - Tokens (B*S = 4096) sharded contiguously: core c owns tokens [512c, 512c+512).
    LayerNorms, QKV projection, attn out-proj, MLP all run on the local 512 tokens.
  - Attention is head-sharded: core c computes heads {2c, 2c+1} over ALL tokens.
    Two AllToAlls exchange (Q^T, K^T, V) token-shards -> head-shards, and the
    attention output O^T head-shards -> token-shards.
  - LayerNorm scale/bias are folded into the following matmul weights on host.
  - Matmuls run as float32r (full PE rate at free-dim >= 256); data stays fp32.
  - Softmax: scores are built transposed S^T[k, q] so exp() output A^T feeds the
    AV matmul directly (lhsT = [V | ones] augmented to also produce the softmax
    sums); normalization by 1/sum is applied on the O^T eviction.
"""

import numpy as np

# ---------------------------------------------------------------- config

B, S, D, H = 2, 2048, 1024, 16
HD = D // H           # 64
FF = 4 * D            # 4096
NC = 8                # cores
TPC = B * S // NC     # 512 tokens per core
EPS = 1e-05

P = 128               # partitions
TT = TPC // P         # 4 token tiles per core
DK = D // P           # 8 contraction tiles over D
FFK = FF // P         # 32 tiles over FF
HPC = H // NC         # 2 heads per core
QB = TPC              # q-block width for attention (= shard width)
NQB = S // QB         # 4 q-blocks per batch
KPB = QB // P         # 4 k-tiles per q-block


def build_program(debug_taps=False, reps=1):
    import contextlib

    import concourse.bass as bass
    import concourse.mybir as mybir
    import concourse.tile as tile
    from concourse import bacc
    from concourse.masks import make_identity, make_upper_triangular

    f32 = mybir.dt.float32
    f32r = mybir.dt.float32r
    AF = mybir.ActivationFunctionType

    nc = bacc.Bacc("TRN2", target_bir_lowering=False, debug=False,
                   num_devices=NC)

    # ---- kernel I/O (per core) ----
    x_d = nc.dram_tensor("x", [TPC, D], f32, kind="ExternalInput").ap()
    caw_d = nc.dram_tensor("c_attn_w", [D, 3 * D], f32r, kind="ExternalInput").ap()
    cab_d = nc.dram_tensor("c_attn_b", [3 * D], f32, kind="ExternalInput").ap()
    cpw_d = nc.dram_tensor("c_proj_w", [D, D], f32r, kind="ExternalInput").ap()
    cpb_d = nc.dram_tensor("c_proj_b", [D], f32, kind="ExternalInput").ap()
    fcw_d = nc.dram_tensor("fc_w", [D, FF], f32r, kind="ExternalInput").ap()
    fcb_d = nc.dram_tensor("fc_b", [FF], f32, kind="ExternalInput").ap()
    pjw_d = nc.dram_tensor("proj_w", [FF, D], f32r, kind="ExternalInput").ap()
    pjb_d = nc.dram_tensor("proj_b", [D], f32, kind="ExternalInput").ap()
    out_d = nc.dram_tensor("out", [TPC, D], f32, kind="ExternalOutput").ap()
    dbg = {}
    if debug_taps:
        dbg["fin"] = nc.dram_tensor("dbg_fin", [NC, 3 * P * TPC], f32r,
                                    kind="ExternalOutput").ap()
        dbg["fout"] = nc.dram_tensor("dbg_fout", [NC, 3 * P * TPC], f32r,
                                     kind="ExternalOutput").ap()
        dbg["bin"] = nc.dram_tensor("dbg_bin", [NC, P * TPC], f32r,
                                    kind="ExternalOutput").ap()
        dbg["bout"] = nc.dram_tensor("dbg_bout", [NC, P * TPC], f32r,
                                     kind="ExternalOutput").ap()

    SLOT = 3 * P * TPC

    with tile.TileContext(nc) as tc:
        ctx = contextlib.ExitStack()
        with ctx:
            dram = ctx.enter_context(tc.tile_pool(name="dram", bufs=1,
                                                  space="DRAM"))
            consts = ctx.enter_context(tc.tile_pool(name="consts", bufs=1))
            stats = ctx.enter_context(tc.tile_pool(name="stats", bufs=2))
            resid = ctx.enter_context(tc.tile_pool(name="resid", bufs=1))
            big = ctx.enter_context(tc.tile_pool(name="big", bufs=1))
            wpool = ctx.enter_context(tc.tile_pool(name="wpool", bufs=6))
            rhsp = ctx.enter_context(tc.tile_pool(name="rhsp", bufs=4))
            temps = ctx.enter_context(tc.tile_pool(name="temps", bufs=4))
            atp = ctx.enter_context(tc.tile_pool(name="atp", bufs=2))
            attin = ctx.enter_context(tc.tile_pool(name="attin", bufs=3))
            psum = ctx.enter_context(tc.tile_pool(name="psum", bufs=1,
                                                  space="PSUM"))

            # fwd slot j: [QT 128xTPC | KT 128xTPC | V TPCx128] for rank j heads
            a2a_fin = dram.tile([NC, SLOT], f32r)
            a2a_fout = dram.tile([NC, SLOT], f32r)
            a2a_bin = dram.tile([NC, P * TPC], f32r)
            a2a_bout = dram.tile([NC, P * TPC], f32r)

            def fwd_in_qt(j):
                return a2a_fin[j, 0:P * TPC].rearrange("(a b) -> a b", b=TPC)

            def fwd_in_kt(j):
                return a2a_fin[j, P * TPC:2 * P * TPC].rearrange(
                    "(a b) -> a b", b=TPC)

            def fwd_in_v(j):
                return a2a_fin[j, 2 * P * TPC:3 * P * TPC].rearrange(
                    "(a b) -> a b", b=P)

            def fwd_out_qt(j):
                return a2a_fout[j, 0:P * TPC].rearrange("(a b) -> a b", b=TPC)

            def fwd_out_kt(j):
                return a2a_fout[j, P * TPC:2 * P * TPC].rearrange(
                    "(a b) -> a b", b=TPC)

            def fwd_out_v(j):
                return a2a_fout[j, 2 * P * TPC:3 * P * TPC].rearrange(
                    "(a b) -> a b", b=P)

            # ---------------- constants ----------------
            ident = consts.tile([P, P], f32)
            make_identity(nc, ident)
            # mask[k, q] = 1 if q >= k else 0 (diagonal 128x128 strips)
            mask_f = consts.tile([P, P], f32)
            make_upper_triangular(nc, mask_f, val=1.0, diag=True)
            mask = consts.tile([P, P], f32r)
            nc.vector.tensor_copy(out=mask, in_=mask_f)
            # f32r-typed constants (memset cannot write f32r directly)
            ones_f = consts.tile([P, 1], f32)
            nc.vector.memset(ones_f, 1.0)
            ones_rr = consts.tile([P, 1], f32r)
            nc.vector.tensor_copy(out=ones_rr, in_=ones_f)
            zero_f = consts.tile([P, (KPB - 1) * P], f32)
            nc.vector.memset(zero_f, 0.0)
            zero_rr = consts.tile([P, (KPB - 1) * P], f32r)
            nc.vector.tensor_copy(out=zero_rr, in_=zero_f)

            cab_qk = consts.tile([P, 2 * DK], f32)   # c_attn_b[0:2D] as [P, 16]
            nc.sync.dma_start(cab_qk, cab_d[0:2 * D].rearrange("(m p) -> p m", p=P))
            fcb_sb = consts.tile([P, FFK], f32)      # fc_b as [P, 32]
            nc.sync.dma_start(fcb_sb, fcb_d.rearrange("(m p) -> p m", p=P))

            def bcast_row(src_ap, off, n):
                t = temps.tile([P, n], f32, tag="ln_out", name="bcast", bufs=2)
                nc.sync.dma_start(t, bass.AP(
                    tensor=src_ap.tensor, offset=src_ap.offset + off,
                    ap=[[0, P], [1, n]]))
                return t

            # ---------------- helpers ----------------
            def ln_transpose(src_tiles, dstT):
                """LayerNorm (ddof=1, eps on std, no scale/bias) each [P, D]
                token tile, then PE-transpose into dstT [P, DK, TPC]."""
                for t in range(TT):
                    xt = src_tiles[t]
                    st = stats.tile([P, 2, nc.vector.BN_STATS_DIM], f32,
                                    tag="bnst")
                    xg = xt.rearrange("p (g d) -> p g d", g=2)
                    for g in range(2):
                        nc.vector.bn_stats(out=st[:, g, :], in_=xg[:, g, :])
                    mv = stats.tile([P, nc.vector.BN_AGGR_DIM], f32, tag="mv")
                    nc.vector.bn_aggr(out=mv, in_=st)
                    sdev = stats.tile([P, 1], f32, tag="sdev")
                    nc.scalar.activation(out=sdev, in_=mv[:, 1:2], func=AF.Sqrt,
                                         scale=float(D) / (D - 1))
                    nc.vector.tensor_scalar_add(sdev, sdev, EPS)
                    rstd = stats.tile([P, 1], f32, tag="rstd")
                    nc.vector.reciprocal(out=rstd, in_=sdev)
                    nmr = stats.tile([P, 1], f32, tag="nmr")
                    nc.vector.tensor_scalar(out=nmr, in0=mv[:, 0:1],
                                            scalar1=rstd, scalar2=-1.0,
                                            op0=mybir.AluOpType.mult,
                                            op1=mybir.AluOpType.mult)
                    xn = temps.tile([P, D], f32, tag="ln_out", bufs=2)
                    nc.scalar.activation(out=xn, in_=xt, func=AF.Identity,
                                         bias=nmr, scale=rstd)
                    for d in range(DK):
                        pt = psum.tile([P, P], f32, tag="tp", bufs=2)
                        nc.tensor.transpose(pt, xn[:, d * P:(d + 1) * P], ident)
                        nc.vector.tensor_copy(out=dstT[:, d, t * P:(t + 1) * P],
                                              in_=pt)

            def whole_block():
                # ---------------- phase 1: load x, LN1 + transpose ----------------
                x_tiles = [resid.tile([P, D], f32, tag=f"x{t}", name=f"x{t}")
                           for t in range(TT)]
                xr = x_d.rearrange("(t p) d -> t p d", p=P)
                for t in range(TT):
                    nc.sync.dma_start(x_tiles[t], xr[t])

                xnT = big.tile([P, DK, TPC], f32r, tag="xT")
                ln_transpose(x_tiles, xnT)

                # ---------------- phase 2: QKV projections ----------------
                for m in range(2 * DK):  # 16 feature tiles: Q then K
                    ps = psum.tile([P, TPC], f32, tag="mmps", bufs=2)
                    for k in range(DK):
                        wt = wpool.tile([P, P], f32r, tag="wlhs")
                        nc.sync.dma_start(wt, caw_d[k * P:(k + 1) * P,
                                                    m * P:(m + 1) * P])
                        nc.tensor.matmul(ps, wt,
                                         xnT[:, k, :],
                                         start=(k == 0), stop=(k == DK - 1))
                    sb = temps.tile([P, TPC], f32r, tag="ev512")
                    nc.scalar.activation(out=sb, in_=ps, func=AF.Identity,
                                         bias=cab_qk[:, m:m + 1])
                    j, half = m % DK, m // DK
                    dst = fwd_in_qt(j) if half == 0 else fwd_in_kt(j)
                    nc.sync.dma_start(dst, sb)

                # V token-major
                vb_bc = bcast_row(cab_d, 2 * D, D)
                for t in range(TT):
                    for nb in range(2):
                        ns = D // 2
                        ps = psum.tile([P, ns], f32, tag="mmps", bufs=2)
                        for k in range(DK):
                            wt = rhsp.tile([P, ns], f32r, tag="wrhs")
                            nc.sync.dma_start(wt, caw_d[k * P:(k + 1) * P,
                                                        2 * D + nb * ns:
                                                        2 * D + (nb + 1) * ns])
                            nc.tensor.matmul(
                                ps, xnT[:, k, t * P:(t + 1) * P],
                                wt,
                                start=(k == 0), stop=(k == DK - 1))
                        sb = temps.tile([P, ns], f32r, tag="ev512")
                        nc.vector.tensor_add(out=sb, in0=ps,
                                             in1=vb_bc[:, nb * ns:(nb + 1) * ns])
                        for jj in range(4):
                            j = nb * 4 + jj
                            nc.sync.dma_start(fwd_in_v(j)[t * P:(t + 1) * P, :],
                                              sb[:, jj * P:(jj + 1) * P])

                # ---------------- phase 3: forward AllToAll ----------------
                if debug_taps:
                    nc.sync.dma_start(dbg["fin"][:], a2a_fin[:])
                nc.gpsimd.collective_compute(
                    "AllToAll", mybir.AluOpType.bypass,
                    replica_groups=[list(range(NC))],
                    ins=[a2a_fin.opt()], outs=[a2a_fout.opt()])
                if debug_taps:
                    nc.sync.dma_start(dbg["fout"][:], a2a_fout[:])

                # ---------------- phase 4: attention (my 2 heads, all tokens) ----
                for b in range(B):
                    for qb in range(NQB):
                        slot_q = b * NQB + qb
                        qt_sb = attin.tile([P, QB], f32r, tag="qt")
                        nc.sync.dma_start(qt_sb, fwd_out_qt(slot_q))
                        opsums = [psum.tile([HD + 1, QB], f32, tag=f"op{h}",
                                            name=f"op{h}", bufs=1)
                                  for h in range(HPC)]
                        nkt = (qb + 1) * KPB
                        for kt in range(nkt):
                            slot_k = b * NQB + kt // KPB
                            off = (kt % KPB) * P
                            kt_sb = attin.tile([P, P], f32r, tag="kt")
                            nc.sync.dma_start(kt_sb,
                                              fwd_out_kt(slot_k)[:, off:off + P])
                            va = attin.tile([P, HPC, HD + 1], f32r, tag="va")
                            nc.sync.dma_start(
                                va[:, :, 0:HD],
                                fwd_out_v(slot_k)[off:off + P, :].rearrange(
                                    "p (h d) -> p h d", h=HPC))
                            for h in range(HPC):
                                nc.vector.tensor_copy(out=va[:, h, HD:HD + 1],
                                                      in_=ones_rr)
                            d = kt - qb * KPB  # >= 0 on diagonal strips
                            for h in range(HPC):
                                sps = psum.tile([P, QB], f32, tag=f"s{h}", bufs=1)
                                nc.tensor.matmul(
                                    sps, kt_sb[h * HD:(h + 1) * HD, :],
                                    qt_sb[h * HD:(h + 1) * HD, :],
                                    start=True, stop=True)
                                at = atp.tile([P, QB], f32r, tag=f"at{h}")
                                if d >= 0:
                                    if d > 0:
                                        nc.vector.tensor_copy(out=at[:, 0:d * P],
                                                              in_=zero_rr[:, 0:d * P])
                                    nc.scalar.activation(
                                        out=at[:, d * P:], in_=sps[:, d * P:],
                                        func=AF.Exp,
                                        scale=1.0 / float(np.sqrt(HD)))
                                    nc.vector.tensor_mul(
                                        out=at[:, d * P:(d + 1) * P],
                                        in0=at[:, d * P:(d + 1) * P], in1=mask)
                                else:
                                    nc.scalar.activation(
                                        out=at, in_=sps, func=AF.Exp,
                                        scale=1.0 / float(np.sqrt(HD)))
                                nc.tensor.matmul(opsums[h],
                                                 va[:, h, :],
                                                 at,
                                                 start=(kt == 0),
                                                 stop=(kt == nkt - 1))
                        # normalize and ship O^T shard to its token-owner rank
                        for h in range(HPC):
                            rs = stats.tile([P, QB], f32, tag="rs")
                            nc.vector.reciprocal(out=rs[HD:HD + 1, :],
                                                 in_=opsums[h][HD:HD + 1, :])
                            rsd = dram.tile([QB], f32, tag="rsd", name="rsd",
                                            bufs=2)
                            nc.sync.dma_start(rsd, rs[HD:HD + 1, :])
                            rbc = stats.tile([HD, QB], f32, tag="rbc")
                            nc.sync.dma_start(rbc, bass.AP(
                                tensor=rsd.tensor, offset=rsd.offset,
                                ap=[[0, HD], [1, QB]]))
                            otv = temps.tile([HD, QB], f32r, tag="ev512")
                            nc.vector.tensor_mul(out=otv, in0=opsums[h][0:HD, :],
                                                 in1=rbc)
                            nc.sync.dma_start(
                                a2a_bin[slot_q,
                                        h * HD * TPC:(h + 1) * HD * TPC].rearrange(
                                            "(a b) -> a b", b=TPC),
                                otv)

                if debug_taps:
                    nc.sync.dma_start(dbg["bin"][:], a2a_bin[:])
                nc.gpsimd.collective_compute(
                    "AllToAll", mybir.AluOpType.bypass,
                    replica_groups=[list(range(NC))],
                    ins=[a2a_bin.opt()], outs=[a2a_bout.opt()])
                if debug_taps:
                    nc.sync.dma_start(dbg["bout"][:], a2a_bout[:])

                # ---------------- phase 5: attn out-proj + residual ----------------
                cpb_bc = bcast_row(cpb_d, 0, D)
                otf = big.tile([P, DK, TPC], f32r, tag="otf")
                nc.sync.dma_start(otf, a2a_bout[:].rearrange(
                    "n (p t) -> p n t", p=P))
                x2_tiles = [resid.tile([P, D], f32, tag=f"x2{t}", name=f"x2{t}")
                            for t in range(TT)]
                for t in range(TT):
                    for nb in range(2):
                        ns = D // 2
                        ps = psum.tile([P, ns], f32, tag="mmps", bufs=2)
                        for k in range(DK):
                            wt = rhsp.tile([P, ns], f32r, tag="wrhs")
                            nc.sync.dma_start(wt, cpw_d[k * P:(k + 1) * P,
                                                        nb * ns:(nb + 1) * ns])
                            nc.tensor.matmul(
                                ps, otf[:, k, t * P:(t + 1) * P],
                                wt,
                                start=(k == 0), stop=(k == DK - 1))
                        sl = slice(nb * ns, (nb + 1) * ns)
                        nc.vector.tensor_add(out=x2_tiles[t][:, sl], in0=ps,
                                             in1=x_tiles[t][:, sl])
                        nc.vector.tensor_add(out=x2_tiles[t][:, sl],
                                             in0=x2_tiles[t][:, sl],
                                             in1=cpb_bc[:, sl])

                # ---------------- phase 6: LN2 + transpose ----------------
                xn2T = big.tile([P, DK, TPC], f32r, tag="xT")
                ln_transpose(x2_tiles, xn2T)

                # ---------------- phase 7: MLP fc + gelu ----------------
                # tanh-approx gelu, exact form: gelu(u) = u * sigmoid(g(u)),
                # g(u) = 2*sqrt(2/pi) * (u + 0.044715 u^3) = K1*u + K2*u^3
                K1 = 2.0 * float(np.sqrt(2.0 / np.pi))
                K2 = K1 * 0.044715
                hT = big.tile([P, FFK, TPC], f32r, tag="hT")
                for m in range(FFK):
                    ps = psum.tile([P, TPC], f32, tag="mmps", bufs=2)
                    for k in range(DK):
                        wt = wpool.tile([P, P], f32r, tag="wlhs")
                        nc.sync.dma_start(wt, fcw_d[k * P:(k + 1) * P,
                                                    m * P:(m + 1) * P])
                        nc.tensor.matmul(ps, wt,
                                         xn2T[:, k, :],
                                         start=(k == 0), stop=(k == DK - 1))
                    u = temps.tile([P, TPC], f32, tag="ev512")
                    nc.scalar.activation(out=u, in_=ps, func=AF.Identity,
                                         bias=fcb_sb[:, m:m + 1])
                    g = temps.tile([P, TPC], f32, tag="ev512")
                    nc.scalar.activation(out=g, in_=u, func=AF.Square,
                                         scale=float(np.sqrt(K2)))
                    nc.vector.tensor_scalar_add(g, g, K1)
                    nc.vector.tensor_mul(out=g, in0=g, in1=u)
                    nc.scalar.activation(out=g, in_=g, func=AF.Sigmoid)
                    nc.vector.tensor_mul(out=hT[:, m, :], in0=g, in1=u)

                # ---------------- phase 8: MLP proj + residual -> out ----------------
                pjb_bc = bcast_row(pjb_d, 0, D)
                outr = out_d.rearrange("(t p) d -> t p d", p=P)
                for t in range(TT):
                    ob = temps.tile([P, D], f32, tag="ln_out", bufs=2)
                    for nb in range(2):
                        ns = D // 2
                        ps = psum.tile([P, ns], f32, tag="mmps", bufs=2)
                        for k in range(FFK):
                            wt = rhsp.tile([P, ns], f32r, tag="wrhs")
                            nc.sync.dma_start(wt, pjw_d[k * P:(k + 1) * P,
                                                        nb * ns:(nb + 1) * ns])
                            nc.tensor.matmul(
                                ps, hT[:, k, t * P:(t + 1) * P],
                                wt,
                                start=(k == 0), stop=(k == FFK - 1))
                        sl = slice(nb * ns, (nb + 1) * ns)
                        nc.vector.tensor_add(out=ob[:, sl], in0=ps,
                                             in1=x2_tiles[t][:, sl])
                        nc.vector.tensor_add(out=ob[:, sl], in0=ob[:, sl],
                                             in1=pjb_bc[:, sl])
                    nc.sync.dma_start(outr[t], ob)


            for _rep in range(reps):
                whole_block()

    nc.compile()
    return nc


_NC_CACHE = None


def _get_program():
    global _NC_CACHE
    if _NC_CACHE is None:
        _NC_CACHE = build_program()
    return _NC_CACHE


def host_fold(inputs):
    """Fold LN scale/bias into the following matmul weights (host side)."""
    def f(a):
        return np.ascontiguousarray(np.asarray(a), dtype=np.float32)
    x = f(inputs["x"]).reshape(B * S, D)
    caw0 = f(inputs["c_attn_w"])
    fcw0 = f(inputs["fc_w"])
    caw = caw0 * f(inputs["ln1_w"])[:, None]
    cab = f(inputs["c_attn_b"]) + f(inputs["ln1_b"]) @ caw0
    fcw = fcw0 * f(inputs["ln2_w"])[:, None]
    fcb = f(inputs["fc_b"]) + f(inputs["ln2_b"]) @ fcw0
    return {
        "x": x,
        "c_attn_w": f(caw), "c_attn_b": f(cab),
        "c_proj_w": f(inputs["c_proj_w"]), "c_proj_b": f(inputs["c_proj_b"]),
        "fc_w": f(fcw), "fc_b": f(fcb),
        "proj_w": f(inputs["proj_w"]), "proj_b": f(inputs["proj_b"]),
    }


def make_in_maps(inputs):
    full = host_fold(inputs)
    in_maps = []
    for c in range(NC):
        m = dict(full)
        m["x"] = np.ascontiguousarray(full["x"][c * TPC:(c + 1) * TPC])
        in_maps.append(m)
    return in_maps


def kernel(**inputs) -> np.ndarray:
    from concourse import bass_utils
    nc = _get_program()
    in_maps = make_in_maps(inputs)
    res = bass_utils.run_bass_kernel_spmd(nc, in_maps, core_ids=list(range(NC)))
    out = np.concatenate([res.results[c]["out"] for c in range(NC)], axis=0)
    return out.reshape(B, S, D)



# revision 21
# speedup vs baseline: 1.1873x; 1.1873x over previous
"""Trainium2 Bass kernel for a GPT-2 style transformer block.

Sharding (8 NeuronCores, SPMD-uniform program):
  - Tokens (B*S = 4096) sharded contiguously: core c owns tokens [512c, 512c+512).
    Residuals, LN2, attn out-proj, MLP run on the local 512 tokens.
  - Attention is head-sharded: core c computes heads {2c, 2c+1} over ALL tokens.
    Instead of an AllToAll to reshard Q/K/V, every core receives the FULL
    input x (replicated) plus a per-core slice of the QKV weights, and
    computes Q/K/V for all 4096 tokens for its own 2 heads locally (same
    total matmul FLOPs as token-sharded QKV: 1/8 of heads instead of 1/8 of
    tokens). LN1 over the full sequence is replicated on every core.
    One backward AllToAll ships O^T head-shards -> token-shards.
  - All matmul operands are bf16 (fp32 PSUM accumulation); residual stream is
    bf16; weights are cast to bf16 on host with LN scale/bias folded in.
  - Weights stream from DRAM in large [128, 8, 512]-shaped strips (one
    dma_start each) through a shared 4-deep window pool.
  - Softmax: scores are built transposed S^T[k, q]; exp() output A^T feeds the
    AV matmul directly (lhsT = [V | ones] augmented to also produce softmax
    sums); 1/sum is broadcast across partitions with a tiny PE outer-product
    and applied on the O^T eviction.
"""

import numpy as np

# ---------------------------------------------------------------- config

B, S, D, H = 2, 2048, 1024, 16
HD = D // H           # 64
FF = 4 * D            # 4096
NC = 8                # cores
TPC = B * S // NC     # 512 tokens per core
EPS = 1e-05

P = 128               # partitions
TT = TPC // P         # 4 token tiles per core
DK = D // P           # 8 contraction tiles over D
FFK = FF // P         # 32 tiles over FF
HPC = H // NC         # 2 heads per core
QB = TPC              # q-block width for attention (= shard width)
NQB = S // QB         # 4 q-blocks per batch
KPB = QB // P         # 4 k-tiles per q-block
NCH = B * S // TPC    # 8 chunks of 512 tokens over the full sequence
MD = HPC * HD         # 128 dims per core (2 heads)

USE_GELU_TANH = True  # single-op ACT gelu vs 5-op sigmoid formulation


def build_program(reps=1):
    import contextlib

    import concourse.bass as bass
    import concourse.mybir as mybir
    import concourse.tile as tile
    from concourse import bacc
    from concourse.masks import make_identity, make_upper_triangular

    f32 = mybir.dt.float32
    bf16 = mybir.dt.bfloat16
    AF = mybir.ActivationFunctionType

    nc = bacc.Bacc("TRN2", target_bir_lowering=False, debug=False,
                   num_devices=NC)

    # ---- kernel I/O (per core) ----
    x_d = nc.dram_tensor("x", [TPC, D], bf16, kind="ExternalInput").ap()
    xf_d = nc.dram_tensor("xf", [B * S, D], bf16, kind="ExternalInput").ap()
    # per-core slice of QKV weights: [D, 3*MD] = my-heads' Q | K | V columns
    caw_d = nc.dram_tensor("c_attn_w", [D, 3 * MD], bf16,
                           kind="ExternalInput").ap()
    cab_d = nc.dram_tensor("c_attn_b", [3 * MD], f32, kind="ExternalInput").ap()
    cpw_d = nc.dram_tensor("c_proj_w", [D, D], bf16, kind="ExternalInput").ap()
    cpb_d = nc.dram_tensor("c_proj_b", [D], f32, kind="ExternalInput").ap()
    fcw_d = nc.dram_tensor("fc_w", [D, FF], bf16, kind="ExternalInput").ap()
    fcb_d = nc.dram_tensor("fc_b", [FF], f32, kind="ExternalInput").ap()
    pjw_d = nc.dram_tensor("proj_w", [FF, D], bf16, kind="ExternalInput").ap()
    pjb_d = nc.dram_tensor("proj_b", [D], f32, kind="ExternalInput").ap()
    out_d = nc.dram_tensor("out", [TPC, D], f32, kind="ExternalOutput").ap()

    PT = P * TPC

    with tile.TileContext(nc) as tc:
        ctx = contextlib.ExitStack()
        with ctx:
            dram = ctx.enter_context(tc.tile_pool(name="dram", bufs=1,
                                                  space="DRAM"))
            consts = ctx.enter_context(tc.tile_pool(name="consts", bufs=1))
            stats = ctx.enter_context(tc.tile_pool(name="stats", bufs=2))
            resid = ctx.enter_context(tc.tile_pool(name="resid", bufs=1))
            big = ctx.enter_context(tc.tile_pool(name="big", bufs=1))
            wpool = ctx.enter_context(tc.tile_pool(name="wpool", bufs=4))
            temps = ctx.enter_context(tc.tile_pool(name="temps", bufs=2))
            attn = ctx.enter_context(tc.tile_pool(name="attn", bufs=2))
            psum = ctx.enter_context(tc.tile_pool(name="psum", bufs=1,
                                                  space="PSUM"))

            a2a_bin = dram.tile([NC, PT], bf16)
            a2a_bout = dram.tile([NC, PT], bf16)
            ag_in = dram.tile([TPC, 2], f32)
            ag_out = dram.tile([B * S, 2], f32)

            # ---------------- constants ----------------
            ident = consts.tile([P, P], bf16)
            make_identity(nc, ident)
            # mask[k, q] = 1 if q >= k else 0 (diagonal 128x128 strips)
            mask = consts.tile([P, P], bf16)
            make_upper_triangular(nc, mask, val=1.0, diag=True)
            cab_sb = consts.tile([P, 3], f32)       # my qkv bias as [P, 3]
            nc.sync.dma_start(cab_sb, cab_d.rearrange("(m p) -> p m", p=P))
            fcb_sb = consts.tile([P, FFK], f32)     # fc_b as [P, 32]
            nc.sync.dma_start(fcb_sb, fcb_d.rearrange("(m p) -> p m", p=P))

            def bcast_row(src_ap, off, n, name):
                t = consts.tile([P, n], f32, name=name)
                nc.sync.dma_start(t, bass.AP(
                    tensor=src_ap.tensor, offset=src_ap.offset + off,
                    ap=[[0, P], [1, n]]))
                return t

            cpb_bc = bcast_row(cpb_d, 0, D, "cpb_bc")
            pjb_bc = bcast_row(pjb_d, 0, D, "pjb_bc")
            vb_bc = bcast_row(cab_d, 2 * MD, MD, "vb_bc")

            # rearranged DRAM views
            cawr = caw_d.rearrange("(k p) n -> p k n", p=P)   # [128, 8, 384]
            cpwr = cpw_d.rearrange("(k p) n -> p k n", p=P)   # [128, 8, 1024]
            fcwr = fcw_d.rearrange("(k p) n -> p k n", p=P)   # [128, 8, 4096]
            pjwr = pjw_d.rearrange("(k p) n -> p k n", p=P)   # [128, 32, 1024]
            xfr = xf_d.rearrange("(c t p) d -> c p t d", p=P, t=TT)

            # ---------------- helpers ----------------
            def ln_stats(xt, shr, t):
                """LayerNorm stats (ddof=1, eps on std) of one [P, D] token
                tile -> shr[:, t, 0] = -mean*rstd, shr[:, t, 1] = rstd.
                Uses Ln+Exp for the sqrt so the ACT table never leaves the
                natural_log_exp set (shared with attention's Exp)."""
                st = stats.tile([P, 2, nc.vector.BN_STATS_DIM], f32,
                                tag="bnst")
                xg = xt.rearrange("p (g d) -> p g d", g=2)
                for g in range(2):
                    nc.vector.bn_stats(out=st[:, g, :], in_=xg[:, g, :])
                mv = stats.tile([P, nc.vector.BN_AGGR_DIM], f32, tag="mv")
                nc.vector.bn_aggr(out=mv, in_=st)
                sdev = stats.tile([P, 1], f32, tag="sdev")
                nc.scalar.activation(out=sdev, in_=mv[:, 1:2], func=AF.Sqrt,
                                     scale=float(D) / (D - 1))
                nc.vector.tensor_scalar_add(sdev, sdev, EPS)
                nc.vector.reciprocal(out=shr[:, t, 1:2], in_=sdev)
                nc.vector.tensor_scalar(out=shr[:, t, 0:1], in0=mv[:, 0:1],
                                        scalar1=shr[:, t, 1:2], scalar2=-1.0,
                                        op0=mybir.AluOpType.mult,
                                        op1=mybir.AluOpType.mult)

            def transpose_block(xn_tiles, dstT):
                """PE-transpose 4 normalized [P, D] tiles into dstT
                [P, DK, TPC] (bf16), batching 4 transposes per PSUM evict."""
                for d in range(DK):
                    pt = psum.tile([P, TPC], bf16, tag="tp", bufs=2,
                                   name="tp")
                    for t in range(TT):
                        nc.tensor.transpose(pt[:, t * P:(t + 1) * P],
                                            xn_tiles[t][:, d * P:(d + 1) * P],
                                            ident)
                    nc.vector.tensor_copy(out=dstT[:, d, :], in_=pt)

            def whole_block():
                # ---- phase 1: stream full x; LN1; X^T; local QKV ----
                qkvw = consts.tile([P, DK, 3 * MD], bf16, name="qkvw")
                nc.sync.dma_start(qkvw, cawr)

                x_tiles = [resid.tile([P, D], bf16, tag=f"x{t}", name=f"x{t}")
                           for t in range(TT)]
                xr = x_d.rearrange("(t p) d -> t p d", p=P)
                for t in range(TT):
                    nc.sync.dma_start(x_tiles[t], xr[t])

                # LN1 stats for OWN tokens only; AllGather the (nmr, rstd)
                # pairs (32 KB) so no core re-computes stats for all tokens.
                shr = temps.tile([P, TT, 2], f32, tag="shr", name="shr")
                for t in range(TT):
                    ln_stats(x_tiles[t], shr, t)
                nc.sync.dma_start(
                    ag_in.rearrange("(t p) s -> p t s", p=P), shr)
                nc.gpsimd.collective_compute(
                    "AllGather", mybir.AluOpType.bypass,
                    replica_groups=[list(range(NC))],
                    ins=[ag_in.opt()], outs=[ag_out.opt()])
                stats_sb = temps.tile([P, NCH * TT, 2], f32, tag="stats_sb",
                                      name="stats_sb")
                nc.sync.dma_start(
                    stats_sb,
                    ag_out.rearrange("(ct p) s -> p ct s", p=P))

                # per-batch attention operand tiles (filled chunk by chunk)
                qtb = [attn.tile([P, NQB, QB], bf16, tag=f"qtb{b}", bufs=1,
                                 name=f"qtb{b}") for b in range(B)]
                ktb = [attn.tile([P, NQB, QB], bf16, tag=f"ktb{b}", bufs=1,
                                 name=f"ktb{b}") for b in range(B)]
                vres = [attn.tile([P, 4 * NQB, HPC, HD + 1], bf16,
                                  tag=f"vres{b}", bufs=1, name=f"vres{b}")
                        for b in range(B)]

                # chunks 0..2 compute LN1 stats locally (redundantly) so the
                # pipeline starts before the AllGather lands; 3..7 use it.
                N_LOCAL_STATS = 3
                for c in range(NCH):
                    b, sl = c // NQB, c % NQB
                    local = c < N_LOCAL_STATS
                    if local:
                        shrc = temps.tile([P, TT, 2], f32, tag="shr",
                                          name="shrc")
                    xn_tiles = []
                    for t in range(TT):
                        xct = temps.tile([P, D], bf16, tag="xct", bufs=3,
                                         name="xct")
                        nc.sync.dma_start(xct, xfr[c][:, t, :])
                        xn = temps.tile([P, D], bf16, tag="ln_out", bufs=4,
                                        name="xn")
                        if local:
                            ln_stats(xct, shrc, t)
                            bias, scale = shrc[:, t, 0:1], shrc[:, t, 1:2]
                        else:
                            ct = c * TT + t
                            bias = stats_sb[:, ct, 0:1]
                            scale = stats_sb[:, ct, 1:2]
                        nc.scalar.activation(out=xn, in_=xct, func=AF.Identity,
                                             bias=bias, scale=scale)
                        xn_tiles.append(xn)
                    xnTc = temps.tile([P, DK, TPC], bf16, tag="xnTc", bufs=2,
                                      name="xnTc")
                    transpose_block(xn_tiles, xnTc)

                    # Q^T, K^T for my heads over this chunk (dim-major)
                    for m in range(2):
                        ps = psum.tile([P, TPC], f32, tag="mmps", bufs=2)
                        for k in range(DK):
                            nc.tensor.matmul(
                                ps, qkvw[:, k, m * MD:(m + 1) * MD],
                                xnTc[:, k, :],
                                start=(k == 0), stop=(k == DK - 1))
                        dst = (qtb if m == 0 else ktb)[b][:, sl, :]
                        nc.scalar.activation(out=dst, in_=ps, func=AF.Identity,
                                             bias=cab_sb[:, m:m + 1])
                    # V token-major for this chunk: [128 tok, 128 dims]
                    for t in range(TT):
                        ps = psum.tile([P, MD], f32, tag="mmps", bufs=2,
                                       name="vps")
                        for k in range(DK):
                            nc.tensor.matmul(
                                ps, xnTc[:, k, t * P:(t + 1) * P],
                                qkvw[:, k, 2 * MD:3 * MD],
                                start=(k == 0), stop=(k == DK - 1))
                        ch = sl * KPB + t
                        nc.vector.tensor_add(
                            out=vres[b][:, ch, :, 0:HD],
                            in0=ps.rearrange("p (h d) -> p h d", h=HPC),
                            in1=vb_bc.rearrange("p (h d) -> p h d", h=HPC))
                for b in range(B):
                    nc.vector.memset(vres[b][:, :, :, HD:HD + 1], 1.0)

                # ---- phase 2: attention (my 2 heads) ----
                for b in range(B):
                    for qb in range(NQB):
                        slot_q = b * NQB + qb
                        opsums = [psum.tile([HD + 1, QB], f32, tag=f"op{h}",
                                            name=f"op{h}", bufs=1)
                                  for h in range(HPC)]
                        nkt = (qb + 1) * KPB
                        sc = 1.0 / float(np.sqrt(HD))
                        for kt in range(nkt):
                            sk, off = kt // KPB, (kt % KPB) * P
                            d = kt - qb * KPB  # >= 0 on diagonal strips
                            lo = max(d, 0) * P
                            for h in range(HPC):
                                sps = psum.tile([P, QB], f32, tag=f"s{h}",
                                                bufs=1, name=f"s{h}")
                                nc.tensor.matmul(
                                    sps[:, lo:],
                                    ktb[b][h * HD:(h + 1) * HD, sk,
                                           off:off + P],
                                    qtb[b][h * HD:(h + 1) * HD, qb, lo:],
                                    start=True, stop=True)
                                at = attn.tile([P, QB], bf16, tag=f"at{h}",
                                               name=f"at{h}")
                                nc.scalar.activation(
                                    out=at[:, lo:], in_=sps[:, lo:],
                                    func=AF.Exp, scale=sc)
                                if d >= 0:
                                    nc.vector.tensor_mul(
                                        out=at[:, d * P:(d + 1) * P],
                                        in0=at[:, d * P:(d + 1) * P],
                                        in1=mask)
                                nc.tensor.matmul(opsums[h][:, lo:],
                                                 vres[b][:, kt, h, :],
                                                 at[:, lo:],
                                                 start=(kt == 0),
                                                 stop=(kt == nkt - 1))
                        # normalize columns by 1/sum and ship O^T shard
                        otv = attn.tile([P, QB], bf16, tag="otv", name="otv")
                        for h in range(HPC):
                            rs = stats.tile([1, QB], bf16, tag="rs")
                            with nc.allow_low_precision(
                                    reason="softmax 1/sum in bf16 is fine at "
                                           "the 2e-2 tolerance"):
                                nc.vector.reciprocal(
                                    out=rs, in_=opsums[h][HD:HD + 1, :])
                            rbc = attn.tile([HD, QB], bf16, tag="rbc",
                                            name="rbc")
                            nc.gpsimd.partition_broadcast(rbc, rs)
                            nc.vector.tensor_mul(
                                out=otv[h * HD:(h + 1) * HD, :],
                                in0=opsums[h][0:HD, :], in1=rbc)
                        nc.sync.dma_start(
                            a2a_bin[slot_q, :].rearrange("(a b) -> a b", b=TPC),
                            otv)

                nc.gpsimd.collective_compute(
                    "AllToAll", mybir.AluOpType.bypass,
                    replica_groups=[list(range(NC))],
                    ins=[a2a_bin.opt()], outs=[a2a_bout.opt()])

                # ---- phase 3: attn out-proj + residual ----
                otf = big.tile([P, DK, TPC], bf16, tag="otf", name="otf")
                nc.sync.dma_start(otf, a2a_bout[:].rearrange(
                    "n (p t) -> p n t", p=P))
                x2_tiles = [resid.tile([P, D], bf16, tag=f"x2{t}",
                                       name=f"x2{t}") for t in range(TT)]
                cpws = []
                for nb in range(2):
                    wg = wpool.tile([P, DK, D // 2], bf16, tag="w",
                                    name="cpwg")
                    nc.sync.dma_start(
                        wg, cpwr[:, :, nb * (D // 2):(nb + 1) * (D // 2)])
                    cpws.append(wg)
                for t in range(TT):
                    for nb in range(2):
                        ns = D // 2
                        ps = psum.tile([P, ns], f32, tag="mmps", bufs=2)
                        for k in range(DK):
                            nc.tensor.matmul(
                                ps, otf[:, k, t * P:(t + 1) * P],
                                cpws[nb][:, k, :],
                                start=(k == 0), stop=(k == DK - 1))
                        sl = slice(nb * ns, (nb + 1) * ns)
                        nc.vector.tensor_add(out=x2_tiles[t][:, sl], in0=ps,
                                             in1=cpb_bc[:, sl])
                        nc.vector.tensor_add(out=x2_tiles[t][:, sl],
                                             in0=x2_tiles[t][:, sl],
                                             in1=x_tiles[t][:, sl])

                # ---- phase 4: LN2 + transpose ----
                shr2 = temps.tile([P, TT, 2], f32, tag="shr", name="shr2")
                xn2 = []
                for t in range(TT):
                    ln_stats(x2_tiles[t], shr2, t)
                    xn = temps.tile([P, D], bf16, tag="ln_out", bufs=4,
                                    name="xn2")
                    nc.scalar.activation(out=xn, in_=x2_tiles[t],
                                         func=AF.Identity,
                                         bias=shr2[:, t, 0:1],
                                         scale=shr2[:, t, 1:2])
                    xn2.append(xn)
                xn2T = big.tile([P, DK, TPC], bf16, tag="xn2T", name="xn2T")
                transpose_block(xn2, xn2T)

                # ---- phase 5: MLP fc + gelu ----
                K1 = 2.0 * float(np.sqrt(2.0 / np.pi))
                K2 = K1 * 0.044715
                hT = big.tile([P, FFK, TPC], bf16, tag="hT", name="hT")
                for g in range(DK):         # 8 fc weight groups of 4 m-tiles
                    wg = wpool.tile([P, DK, 4 * P], bf16, tag="w", name="fcwg")
                    nc.sync.dma_start(
                        wg, fcwr[:, :, g * 4 * P:(g + 1) * 4 * P])
                    for mt in range(4):
                        m = g * 4 + mt
                        ps = psum.tile([P, TPC], f32, tag="mmps", bufs=2)
                        for k in range(DK):
                            nc.tensor.matmul(
                                ps, wg[:, k, mt * P:(mt + 1) * P],
                                xn2T[:, k, :],
                                start=(k == 0), stop=(k == DK - 1))
                        if USE_GELU_TANH:
                            nc.scalar.activation(
                                out=hT[:, m, :], in_=ps,
                                func=AF.Gelu_apprx_tanh,
                                bias=fcb_sb[:, m:m + 1])
                        else:
                            u = temps.tile([P, TPC], f32, tag="gelu_u",
                                           name="u")
                            nc.scalar.activation(out=u, in_=ps,
                                                 func=AF.Identity,
                                                 bias=fcb_sb[:, m:m + 1])
                            gt = temps.tile([P, TPC], f32, tag="gelu_g",
                                            name="gt")
                            nc.scalar.activation(out=gt, in_=u,
                                                 func=AF.Square,
                                                 scale=float(np.sqrt(K2)))
                            nc.vector.tensor_scalar_add(gt, gt, K1)
                            nc.vector.tensor_mul(out=gt, in0=gt, in1=u)
                            nc.scalar.activation(out=gt, in_=gt,
                                                 func=AF.Sigmoid)
                            nc.vector.tensor_mul(out=hT[:, m, :], in0=gt,
                                                 in1=u)

                # ---- phase 6: MLP proj + residual -> out ----
                # 8 live PSUM banks (t, nb); stream proj weights by k-groups.
                pj_tags = ["tp", "tp", "mmps", "mmps", "s0", "s1", "op0",
                           "op1"]
                pj_bufs = [2, 2, 2, 2, 1, 1, 1, 1]
                pj_ps = {}
                for i, (t, nb) in enumerate([(t, nb) for t in range(TT)
                                             for nb in range(2)]):
                    pj_ps[(t, nb)] = psum.tile(
                        [P, D // 2], f32, tag=pj_tags[i], bufs=pj_bufs[i],
                        name=f"pj{t}{nb}")
                for g in range(DK):         # 8 pjw groups of 4 k-strips
                    wg = wpool.tile([P, 4, D], bf16, tag="w", name="pjwg")
                    nc.sync.dma_start(wg, pjwr[:, g * 4:(g + 1) * 4, :])
                    for kk in range(4):
                        k = g * 4 + kk
                        for t in range(TT):
                            for nb in range(2):
                                ns = D // 2
                                nc.tensor.matmul(
                                    pj_ps[(t, nb)],
                                    hT[:, k, t * P:(t + 1) * P],
                                    wg[:, kk, nb * ns:(nb + 1) * ns],
                                    start=(k == 0), stop=(k == FFK - 1))
                outr = out_d.rearrange("(t p) d -> t p d", p=P)
                for t in range(TT):
                    ob = temps.tile([P, D], f32, tag="ob", name="ob")
                    for nb in range(2):
                        ns = D // 2
                        sl = slice(nb * ns, (nb + 1) * ns)
                        nc.vector.tensor_add(out=ob[:, sl],
                                             in0=pj_ps[(t, nb)],
                                             in1=pjb_bc[:, sl])
                        nc.vector.tensor_add(out=ob[:, sl], in0=ob[:, sl],
                                             in1=x2_tiles[t][:, sl])
                    nc.sync.dma_start(outr[t], ob)

            for _rep in range(reps):
                whole_block()

    nc.compile()
    return nc


_NC_CACHE = None


def _get_program():
    global _NC_CACHE
    if _NC_CACHE is None:
        _NC_CACHE = build_program()
    return _NC_CACHE


def host_fold(inputs):
    """Fold LN scale/bias into the following matmul weights; cast matmul
    operands to bf16 (host side)."""
    import ml_dtypes
    bf16 = ml_dtypes.bfloat16

    def f(a):
        return np.ascontiguousarray(np.asarray(a), dtype=np.float32)

    x = f(inputs["x"]).reshape(B * S, D)
    caw0 = f(inputs["c_attn_w"])
    fcw0 = f(inputs["fc_w"])
    caw = caw0 * f(inputs["ln1_w"])[:, None]
    cab = f(inputs["c_attn_b"]) + f(inputs["ln1_b"]) @ caw0
    fcw = fcw0 * f(inputs["ln2_w"])[:, None]
    fcb = f(inputs["fc_b"]) + f(inputs["ln2_b"]) @ fcw0

    def h(a):
        return np.ascontiguousarray(np.asarray(a, dtype=bf16))

    return {
        "x": h(x), "caw": f(caw), "cab": f(cab),
        "c_proj_w": h(inputs["c_proj_w"]), "c_proj_b": f(inputs["c_proj_b"]),
        "fc_w": h(fcw), "fc_b": f(fcb),
        "proj_w": h(inputs["proj_w"]), "proj_b": f(inputs["proj_b"]),
    }


def make_in_maps(inputs):
    import ml_dtypes
    bf16 = ml_dtypes.bfloat16
    full = host_fold(inputs)
    xf = full["x"]
    in_maps = []
    for c in range(NC):
        cs, ce = c * MD, (c + 1) * MD
        caw_mine = np.concatenate(
            [full["caw"][:, cs:ce], full["caw"][:, D + cs:D + ce],
             full["caw"][:, 2 * D + cs:2 * D + ce]], axis=1)
        cab_mine = np.concatenate(
            [full["cab"][cs:ce], full["cab"][D + cs:D + ce],
             full["cab"][2 * D + cs:2 * D + ce]])
        m = {
            "x": np.ascontiguousarray(xf[c * TPC:(c + 1) * TPC]),
            "xf": xf,
            "c_attn_w": np.ascontiguousarray(caw_mine.astype(bf16)),
            "c_attn_b": np.ascontiguousarray(cab_mine, dtype=np.float32),
            "c_proj_w": full["c_proj_w"], "c_proj_b": full["c_proj_b"],
            "fc_w": full["fc_w"], "fc_b": full["fc_b"],
            "proj_w": full["proj_w"], "proj_b": full["proj_b"],
        }
        in_maps.append(m)
    return in_maps


def kernel(**inputs) -> np.ndarray:
    from concourse import bass_utils
    nc = _get_program()
    in_maps = make_in_maps(inputs)
    res = bass_utils.run_bass_kernel_spmd(nc, in_maps, core_ids=list(range(NC)))
    out = np.concatenate([res.results[c]["out"] for c in range(NC)], axis=0)
    return np.asarray(out, dtype=np.float32).reshape(B, S, D)
